# revision 11
# baseline (speedup 1.0000x reference)
"""nn_AxialAttention — full-input contract, 8-core TRN2 Bass kernel.

Sharding: core = b*4 + axis*2 + d. Each core computes its (b, axis, d)
rep end-to-end: projections (token-major), RoPE rotation, PE-transpose to
channel-major, sigmoid attention along its axis, output projection, and
an indirect-DMA scatter of its partial (canonical token order) into DRAM.
A ReduceScatter over each batch's 4 cores sums partials; each core
returns one quarter of its batch's output rows. Host concatenates.

Self-contained: shapes hardcoded; no sibling imports.
"""
import os
import numpy as np
import ml_dtypes

import jax
from jax.sharding import Mesh, PartitionSpec, NamedSharding
from jax.experimental.shard_map import shard_map

import concourse.bacc as bacc
import concourse.bass as bass
import concourse.mybir as mybir
import concourse.tile as tile
from concourse import bass2jax
from concourse.masks import make_identity

# ---- problem dims ----
B, Y, X = 2, 64, 64
L, NL, T = 64, 64, 4096
CI, CF, F = 512, 256, 4
NH, G = 8, 2
M = NH // G
HI, HF = 32, 16
VHI, VHF = 64, 32
NHEADS = 16                       # (c,m,g) per core
SCALE = float(1.0 / np.sqrt(2 * HI + F * 2 * HF))

N_CORES = 8
TB = 256                          # tokens per block (4 lines)
NBLK = T // TB                    # 16
TT = TB // 128                    # token-tiles per block = 2
LPB = TB // L                     # lines per block = 4

bf16 = mybir.dt.bfloat16
i32 = mybir.dt.int32
f32 = mybir.dt.float32
nbf = ml_dtypes.bfloat16


# =====================================================================
# Bass program
# =====================================================================

def build_nc(n_blocks=NBLK):
    nc = bacc.Bacc(None, target_bir_lowering=False)

    dp = lambda n, s, dt=bf16: nc.declare_dram_parameter(n, list(s), dt, isOutput=False)
    xi_d = dp("xi_t", (CI, T))
    xf_d = dp("xf_t", (F * CF, T))
    wqi_d = dp("wqi", (CI, 1024))
    wqf_d = dp("wqf", (CF, 512))
    wki_d = dp("wki", (CI, 256))
    wkf_d = dp("wkf", (CF, 128))
    wvi_d = dp("wvi", (CI, 256))
    wvf_d = dp("wvf", (CF, 128))
    woi_d = dp("wo_i", (1024, 512))
    wof_d = dp("wo_f", (512, 256))
    cosi_d = dp("cos_i", (128, 512))
    sini_d = dp("sin_i", (128, 512))
    cosf_d = dp("cos_f", (128, 1024))
    sinf_d = dp("sin_f", (128, 1024))
    mask_d = dp("maskrep", (128, 64))
    bv_d = dp("bvrep", (128, 768))
    pidx_d = dp("pidx", (128, 32), i32)

    full = n_blocks == NBLK
    if full:
        out_ext = nc.declare_dram_parameter("out", [T // 4, 1536], bf16, isOutput=True)
    else:
        out_ext = nc.declare_dram_parameter("out", [T, 1536], bf16, isOutput=True)

    from contextlib import ExitStack
    with tile.TileContext(nc) as tc, ExitStack() as es:
        cst = es.enter_context(tc.tile_pool(name="cst", bufs=1))
        xp = es.enter_context(tc.tile_pool(name="xp", bufs=2))
        rawp = es.enter_context(tc.tile_pool(name="rawp", bufs=1))
        rotp = es.enter_context(tc.tile_pool(name="rotp", bufs=1))
        tmpp = es.enter_context(tc.tile_pool(name="tmpp", bufs=2))
        cmp_ = es.enter_context(tc.tile_pool(name="cmp", bufs=1))
        attp = es.enter_context(tc.tile_pool(name="attp", bufs=1))
        outp = es.enter_context(tc.tile_pool(name="outp", bufs=2))
        pp = es.enter_context(tc.tile_pool(name="pp", bufs=2, space="PSUM"))
        dramp = es.enter_context(tc.tile_pool(name="dramp", bufs=1, space="DRAM"))

        # ---- constants into SBUF ----
        def load_w(dram, cols, ktiles):
            t = cst.tile([128, ktiles, cols], bf16, tag=dram.name + "_sb")
            for k in range(ktiles):
                nc.sync.dma_start(out=t[:, k, :], in_=dram[k * 128:(k + 1) * 128, :])
            return t

        wqi = load_w(wqi_d, 1024, 4)
        wqf = load_w(wqf_d, 512, 2)
        wki = load_w(wki_d, 256, 4)
        wkf = load_w(wkf_d, 128, 2)
        wvi = load_w(wvi_d, 256, 4)
        wvf = load_w(wvf_d, 128, 2)
        woi = load_w(woi_d, 512, 8)
        wof = load_w(wof_d, 256, 4)

        def load_t(dram, shape, dt=bf16):
            t = cst.tile(list(shape), dt, tag=dram.name + "_sb")
            nc.sync.dma_start(out=t[:], in_=dram[:])
            return t

        cosi = load_t(cosi_d, (128, 512))
        sini = load_t(sini_d, (128, 512))
        cosf = load_t(cosf_d, (128, 1024))
        sinf = load_t(sinf_d, (128, 1024))
        maskr = load_t(mask_d, (128, 64))
        bvr = load_t(bv_d, (128, 768))
        pidx = load_t(pidx_d, (128, 32), i32)
        ident = cst.tile([128, 128], bf16)
        make_identity(nc, ident[:])

        # ---- DRAM intermediates ----
        partial = dramp.tile([T, 1536], bf16)
        if full:
            rsout = dramp.tile([T // 4, 1536], bf16)
        else:
            zt = cst.tile([128, 1536], bf16)
            nc.vector.memset(zt[:], 0.0)
            for i in range(T // 128):
                nc.sync.dma_start(out=partial[i * 128:(i + 1) * 128, :], in_=zt[:])

        def ap_ins(apx, pos, step, count):
            """Insert a [step, count] dim at free position pos (0-based after
            partition dim)."""
            lst = [list(d) for d in apx.ap]
            lst.insert(1 + pos, [step, count])
            return bass.AP(tensor=apx.tensor, offset=apx.offset, ap=lst)

        for blk in range(n_blocks):
            c0 = blk * TB          # first token of block
            l0 = blk * LPB         # first line

            # ============== load x ==============
            xi = xp.tile([128, 4, TB], bf16)
            xf = xp.tile([128, 8, TB], bf16)
            for k in range(4):
                nc.sync.dma_start(out=xi[:, k, :], in_=xi_d[k * 128:(k + 1) * 128, c0:c0 + TB])
            for k in range(8):
                nc.sync.dma_start(out=xf[:, k, :], in_=xf_d[k * 128:(k + 1) * 128, c0:c0 + TB])

            # ============== projections (token-major) ==============
            qi = rawp.tile([128, TT, 1024], bf16)
            qf = rawp.tile([128, TT, 2048], bf16)
            ki = rawp.tile([128, TT, 256], bf16)
            kf = rawp.tile([128, TT, 512], bf16)
            v = rawp.tile([128, TT, 768], bf16)

            for tt in range(TT):
                xsl = lambda k: xi[:, k, tt * 128:(tt + 1) * 128]
                xfs = lambda f, k: xf[:, f * 2 + k, tt * 128:(tt + 1) * 128]

                # q_i: (128 tok, 1024) = 2 psum banks
                for half in range(2):
                    ps_qi = pp.tile([128, 512], f32, tag="psA")
                    for k in range(4):
                        nc.tensor.matmul(ps_qi[:], xsl(k),
                                         wqi[:, k, half * 512:(half + 1) * 512],
                                         start=(k == 0), stop=(k == 3))
                    nc.any.tensor_copy(qi[:, tt, half * 512:(half + 1) * 512], ps_qi[:])

                # k_i (256) + v_i (256) share a bank each
                ps_ki = pp.tile([128, 256], f32, tag="psB")
                for k in range(4):
                    nc.tensor.matmul(ps_ki[:], xsl(k), wki[:, k, :],
                                     start=(k == 0), stop=(k == 3))
                nc.any.tensor_copy(ki[:, tt, :], ps_ki[:])

                ps_vi = pp.tile([128, 256], f32, tag="psB")
                for k in range(4):
                    nc.tensor.matmul(ps_vi[:], xsl(k), wvi[:, k, :],
                                     start=(k == 0), stop=(k == 3))
                # v cols (c,g,[vi 64|vf 128]): vi part strided + bias
                v_r = v[:, tt, :].rearrange("p (cg d) -> p cg d", cg=4)
                nc.vector.tensor_add(v_r[:, :, 0:64],
                                     ps_vi[:].rearrange("p (cg d) -> p cg d", cg=4),
                                     bvr[:].rearrange("p (cg d) -> p cg d", cg=4)[:, :, 0:64])

                for f in range(F):
                    # q_f per flavour: compact psum (128, 512) cols (c,m,g,p,h)
                    ps_qf = pp.tile([128, 512], f32, tag="psA")
                    for k in range(2):
                        nc.tensor.matmul(ps_qf[:], xfs(f, k), wqf[:, k, :],
                                         start=(k == 0), stop=(k == 1))
                    # scatter into qf cols (c,m,g,p,f,h): dims (cmgp 32, f, h)
                    qf_r = qf[:, tt, :].rearrange("p (a f h) -> p a f h", f=F, h=HF)
                    nc.any.tensor_copy(
                        qf_r[:, :, f, :],
                        ps_qf[:].rearrange("p (a h) -> p a h", h=HF))

                    ps_kf = pp.tile([128, 128], f32, tag="psC")
                    for k in range(2):
                        nc.tensor.matmul(ps_kf[:], xfs(f, k), wkf[:, k, :],
                                         start=(k == 0), stop=(k == 1))
                    kf_r = kf[:, tt, :].rearrange("p (a f h) -> p a f h", f=F, h=HF)
                    nc.any.tensor_copy(
                        kf_r[:, :, f, :],
                        ps_kf[:].rearrange("p (a h) -> p a h", h=HF))

                    ps_vf = pp.tile([128, 128], f32, tag="psC")
                    for k in range(2):
                        nc.tensor.matmul(ps_vf[:], xfs(f, k), wvf[:, k, :],
                                         start=(k == 0), stop=(k == 1))
                    # v cols (c,g, 64 + f*32 + hv)
                    nc.vector.tensor_add(
                        v_r[:, :, 64 + f * 32:64 + (f + 1) * 32],
                        ps_vf[:].rearrange("p (cg d) -> p cg d", cg=4),
                        bvr[:].rearrange("p (cg d) -> p cg d", cg=4)[:, :, 64 + f * 32:64 + (f + 1) * 32])

            # ============== RoPE rotation (token-major, in-place for q) ====
            kri = rotp.tile([128, TT, 1024], bf16)
            krf = rotp.tile([128, TT, 2048], bf16)

            for tt in range(TT):
                # --- q_i in-place: cols (c,m,g,p,h) ---
                qir = qi[:, tt, :].rearrange("p (a pp h) -> p a pp h", pp=2, h=HI)
                ci_r = cosi[:].rearrange("p (a h) -> p a h", h=HI)
                si_r = sini[:].rearrange("p (a h) -> p a h", h=HI)
                t0 = tmpp.tile([128, 512], bf16, tag="t0")
                t1 = tmpp.tile([128, 512], bf16, tag="t1")
                t2 = tmpp.tile([128, 512], bf16, tag="t2")
                t0r = t0[:].rearrange("p (a h) -> p a h", h=HI)
                t1r = t1[:].rearrange("p (a h) -> p a h", h=HI)
                t2r = t2[:].rearrange("p (a h) -> p a h", h=HI)
                nc.vector.tensor_mul(t0r, qir[:, :, 0, :], ci_r)   # cos*p0
                nc.vector.tensor_mul(t1r, qir[:, :, 0, :], si_r)   # sin*p0
                nc.vector.tensor_mul(t2r, qir[:, :, 1, :], si_r)   # sin*p1
                nc.vector.tensor_add(qir[:, :, 0, :], t0r, t2r)
                nc.vector.tensor_mul(t0r, qir[:, :, 1, :], ci_r)   # cos*p1
                nc.vector.tensor_sub(qir[:, :, 1, :], t0r, t1r)

                # --- q_f in-place: cols (c,m,g,p,f,h) ---
                qfr = qf[:, tt, :].rearrange("p (a pp fh) -> p a pp fh", pp=2, fh=64)
                cf_r = cosf[:].rearrange("p (a fh) -> p a fh", fh=64)
                sf_r = sinf[:].rearrange("p (a fh) -> p a fh", fh=64)
                u0 = tmpp.tile([128, 1024], bf16, tag="u0")
                u1 = tmpp.tile([128, 1024], bf16, tag="u1")
                u2 = tmpp.tile([128, 1024], bf16, tag="u2")
                u0r = u0[:].rearrange("p (a fh) -> p a fh", fh=64)
                u1r = u1[:].rearrange("p (a fh) -> p a fh", fh=64)
                u2r = u2[:].rearrange("p (a fh) -> p a fh", fh=64)
                nc.vector.tensor_mul(u0r, qfr[:, :, 0, :], cf_r)
                nc.vector.tensor_mul(u1r, qfr[:, :, 0, :], sf_r)
                nc.vector.tensor_mul(u2r, qfr[:, :, 1, :], sf_r)
                nc.vector.tensor_add(qfr[:, :, 0, :], u0r, u2r)
                nc.vector.tensor_mul(u0r, qfr[:, :, 1, :], cf_r)
                nc.vector.tensor_sub(qfr[:, :, 1, :], u0r, u1r)

                # --- k_i -> kri (m-broadcast): out cols (c,m,g,p,h) ---
                # split per c: walrus allows at most 3 free AP dims
                kio = kri[:, tt, :].rearrange("p (c m g pp h) -> p c m g pp h",
                                              c=2, m=M, g=G, pp=2)
                cik = cosi[:].rearrange("p (c m g h) -> p c m g h", c=2, m=M, g=G)
                sik = sini[:].rearrange("p (c m g h) -> p c m g h", c=2, m=M, g=G)
                kis = ki[:, tt, :].rearrange("p (c g pp h) -> p c g pp h", c=2, g=G, pp=2)
                w0 = tmpp.tile([128, 512], bf16, tag="w0")
                w1 = tmpp.tile([128, 512], bf16, tag="w1")
                w0r = w0[:].rearrange("p (c m g h) -> p c m g h", c=2, m=M, g=G)
                w1r = w1[:].rearrange("p (c m g h) -> p c m g h", c=2, m=M, g=G)
                for c in range(2):
                    ki0 = ap_ins(kis[:, c, :, 0, :], 0, 0, M)   # (m0, g, h)
                    ki1 = ap_ins(kis[:, c, :, 1, :], 0, 0, M)
                    nc.vector.tensor_mul(w0r[:, c], ki0, cik[:, c])
                    nc.vector.tensor_mul(w1r[:, c], ki1, sik[:, c])
                    nc.vector.tensor_add(kio[:, c, :, :, 0, :], w0r[:, c], w1r[:, c])
                    nc.vector.tensor_mul(w0r[:, c], ki1, cik[:, c])
                    nc.vector.tensor_mul(w1r[:, c], ki0, sik[:, c])
                    nc.vector.tensor_sub(kio[:, c, :, :, 1, :], w0r[:, c], w1r[:, c])

                # --- k_f -> krf: out cols (c,m,g,p,f,h) ---
                kfo = krf[:, tt, :].rearrange("p (c m g pp fh) -> p c m g pp fh",
                                              c=2, m=M, g=G, pp=2)
                cfk = cosf[:].rearrange("p (c m g fh) -> p c m g fh", c=2, m=M, g=G)
                sfk = sinf[:].rearrange("p (c m g fh) -> p c m g fh", c=2, m=M, g=G)
                kfs = kf[:, tt, :].rearrange("p (c g pp fh) -> p c g pp fh", c=2, g=G, pp=2)
                y0 = tmpp.tile([128, 1024], bf16, tag="y0")
                y1 = tmpp.tile([128, 1024], bf16, tag="y1")
                y0r = y0[:].rearrange("p (c m g fh) -> p c m g fh", c=2, m=M, g=G)
                y1r = y1[:].rearrange("p (c m g fh) -> p c m g fh", c=2, m=M, g=G)
                for c in range(2):
                    kf0 = ap_ins(kfs[:, c, :, 0, :], 0, 0, M)
                    kf1 = ap_ins(kfs[:, c, :, 1, :], 0, 0, M)
                    nc.vector.tensor_mul(y0r[:, c], kf0, cfk[:, c])
                    nc.vector.tensor_mul(y1r[:, c], kf1, sfk[:, c])
                    nc.vector.tensor_add(kfo[:, c, :, :, 0, :], y0r[:, c], y1r[:, c])
                    nc.vector.tensor_mul(y0r[:, c], kf1, cfk[:, c])
                    nc.vector.tensor_mul(y1r[:, c], kf0, sfk[:, c])
                    nc.vector.tensor_sub(kfo[:, c, :, :, 1, :], y0r[:, c], y1r[:, c])

            # ============== transpose to channel-major ==============
            Qi = cmp_.tile([128, 8, TB], bf16)
            Qf = cmp_.tile([128, 16, TB], bf16)
            Ki = cmp_.tile([128, 8, TB], bf16)
            Kf = cmp_.tile([128, 16, TB], bf16)

            def transp(dst, src, nct):
                # src: raw tile (128, TT, nct*128) token-major
                for ct in range(nct):
                    ps_t = pp.tile([128, TT * 128], bf16, tag="psT")
                    for tt in range(TT):
                        nc.tensor.transpose(
                            ps_t[:, tt * 128:(tt + 1) * 128],
                            src[:, tt, ct * 128:(ct + 1) * 128],
                            ident[:])
                    nc.any.tensor_copy(dst[:, ct, :], ps_t[:])

            transp(Qi, qi, 8)
            transp(Qf, qf, 16)
            transp(Ki, kri, 8)
            transp(Kf, krf, 16)

            # ============== attention ==============
            # base-0 copies for matmul operand base alignment
            vline = attp.tile([64, LPB, 768], bf16)
            for l in range(LPB):
                nc.any.tensor_copy(vline[:, l, :],
                                   v[(l % 2) * 64:(l % 2) * 64 + 64, l // 2, :])

            wt = attp.tile([64, 16, LPB * 64], bf16)     # per head, base 0
            for hp in range(8):                           # head pairs
                ps_lg = pp.tile([128, LPB * 64], f32, tag="psB")
                for hh in range(2):
                    hd = hp * 2 + hh
                    for l in range(LPB):
                        qsl = slice(l * 64, (l + 1) * 64)
                        nc.tensor.matmul(
                            ps_lg[hh * 64:(hh + 1) * 64, qsl],
                            Ki[(hd % 2) * 64:(hd % 2) * 64 + 64, hd // 2, qsl],
                            Qi[(hd % 2) * 64:(hd % 2) * 64 + 64, hd // 2, qsl],
                            start=True, stop=False)
                        nc.tensor.matmul(
                            ps_lg[hh * 64:(hh + 1) * 64, qsl],
                            Kf[:, hd, qsl], Qf[:, hd, qsl],
                            start=False, stop=True)
                # sigmoid(scale*logits) then mask (k-token rows, per line)
                msl = maskr[:, l0:l0 + LPB]
                mbc = bass.AP(tensor=msl.tensor, offset=msl.offset,
                              ap=[[msl.ap[0][0], 64], list(msl.ap[1]), [0, 64]])
                for hh in range(2):
                    hd = hp * 2 + hh
                    nc.scalar.activation(out=wt[:, hd, :],
                                         in_=ps_lg[hh * 64:(hh + 1) * 64, :],
                                         func=mybir.ActivationFunctionType.Sigmoid,
                                         scale=SCALE)
                    nc.vector.tensor_mul(
                        wt[:, hd, :].rearrange("p (l q) -> p l q", q=64),
                        wt[:, hd, :].rearrange("p (l q) -> p l q", q=64),
                        mbc)

            avi = attp.tile([128, 8, TB], bf16)          # rows (hd%2, hv64)
            avf = attp.tile([128, 16, TB], bf16)         # f-major: tile f*4+hd//4
            for hp in range(8):
                ps_avi = pp.tile([128, LPB * 64], f32, tag="psB")
                for hh in range(2):
                    hd = hp * 2 + hh
                    c_, mg = divmod(hd, 8)
                    g_ = mg % 2
                    cg = c_ * G + g_
                    for l in range(LPB):
                        qsl = slice(l * 64, (l + 1) * 64)
                        nc.tensor.matmul(
                            ps_avi[hh * 64:(hh + 1) * 64, qsl],
                            vline[:, l, cg * 192:cg * 192 + 64],
                            wt[:, hd, qsl],
                            start=True, stop=True)
                nc.any.tensor_copy(avi[:, hp, :], ps_avi[:])

            for hd in range(NHEADS):
                c_, mg = divmod(hd, 8)
                g_ = mg % 2
                cg = c_ * G + g_
                ps_avf = pp.tile([128, LPB * 64], f32, tag="psB")
                for l in range(LPB):
                    qsl = slice(l * 64, (l + 1) * 64)
                    nc.tensor.matmul(
                        ps_avf[:, qsl],
                        vline[:, l, cg * 192 + 64:cg * 192 + 192],
                        wt[:, hd, qsl],
                        start=True, stop=True)
                # f-regroup: psum rows (f,hv) -> avf[(hd%4)*32:+32, f*4+hd//4]
                for f in range(F):
                    nc.any.tensor_copy(
                        avf[(hd % 4) * 32:(hd % 4) * 32 + 32, f * 4 + hd // 4, :],
                        ps_avf[f * 32:(f + 1) * 32, :])

            # ============== output projection ==============
            for tt in range(TT):
                tsl = slice(tt * 128, (tt + 1) * 128)
                out_sb = outp.tile([128, 1536], bf16, tag="out_sb")
                ps_oi = pp.tile([128, 512], f32, tag="psA")
                for kt in range(8):
                    nc.tensor.matmul(ps_oi[:], avi[:, kt, tsl], woi[:, kt, :],
                                     start=(kt == 0), stop=(kt == 7))
                nc.any.tensor_copy(out_sb[:, 0:512], ps_oi[:])
                for fp in range(2):                      # f pairs
                    ps_of = pp.tile([128, 512], f32, tag="psA")
                    for fi in range(2):
                        f = fp * 2 + fi
                        for q4 in range(4):
                            nc.tensor.matmul(
                                ps_of[:, fi * 256:(fi + 1) * 256],
                                avf[:, f * 4 + q4, tsl], wof[:, q4, :],
                                start=(q4 == 0), stop=(q4 == 3))
                    nc.any.tensor_copy(out_sb[:, 512 + fp * 512:1024 + fp * 512], ps_of[:])

                tg = blk * TT + tt
                nc.gpsimd.indirect_dma_start(
                    out=partial[:],
                    out_offset=bass.IndirectOffsetOnAxis(ap=pidx[:, tg:tg + 1], axis=0),
                    in_=out_sb[:],
                    in_offset=None)

        # ============== collective + output ==============
        if full:
            nc.gpsimd.collective_compute(
                "ReduceScatter", mybir.AluOpType.add,
                replica_groups=[[0, 1, 2, 3], [4, 5, 6, 7]],
                ins=[partial.opt()], outs=[rsout.opt()])
            nc.sync.dma_start(out=out_ext[:], in_=rsout[:])
        else:
            nc.sync.dma_start(out=out_ext[:], in_=partial[:])

    nc.finalize()
    return nc


# =====================================================================
# Host-side prep (mirrors emu.py, bf16)
# =====================================================================

def _rope_scaling(h):
    return np.pi / np.array([np.linspace(1, 30, h), np.linspace(0.1, 1, h)],
                            dtype=np.float32).T


def host_prep(inputs, core):
    f32n = np.float32
    b, rest = divmod(core, 4)
    axis, d = divmod(rest, 2)
    a = axis * 2 + d
    sgn = 1.0 if d == 0 else -1.0

    x_inv = np.asarray(inputs["x_inv"], f32n)[b]
    x_fl = np.asarray(inputs["x_fl"], f32n)[b]
    if axis == 0:
        x_inv = x_inv.transpose(1, 0, 2)
        x_fl = x_fl.transpose(1, 0, 2, 3)
    xi_t = np.ascontiguousarray(x_inv.reshape(T, CI).T.astype(nbf))
    xf_t = np.ascontiguousarray(x_fl.reshape(T, F * CF).T.astype(nbf))

    def perm_q(W, h_):
        cin = W.shape[0]
        Wr = W.reshape(cin, 2, M, G, h_, 2)
        return np.ascontiguousarray(
            Wr.transpose(0, 1, 2, 3, 5, 4).reshape(cin, -1).astype(nbf))

    def perm_k(W, h_):
        cin = W.shape[0]
        Wr = W.reshape(cin, 2, G, h_, 2)
        return np.ascontiguousarray(
            Wr.transpose(0, 1, 2, 4, 3).reshape(cin, -1).astype(nbf))

    wqi = perm_q(np.asarray(inputs["Wq_inv"], f32n)[:, a], HI)
    wqf = perm_q(np.asarray(inputs["Wq_fl"], f32n)[:, a], HF)
    wki = perm_k(np.asarray(inputs["Wk_inv"], f32n)[:, a], HI)
    wkf = perm_k(np.asarray(inputs["Wk_fl"], f32n)[:, a], HF)
    wvi = np.ascontiguousarray(np.asarray(inputs["Wv_inv"], f32n)[:, a]
                               .reshape(CI, -1).astype(nbf))
    wvf = np.ascontiguousarray(np.asarray(inputs["Wv_fl"], f32n)[:, a]
                               .reshape(CF, -1).astype(nbf))
    wo_i = np.ascontiguousarray(np.asarray(inputs["Wo_inv"], f32n)[a]
                                .reshape(1024, CI).astype(nbf))
    wo_f = np.ascontiguousarray(np.asarray(inputs["Wo_fl"], f32n)[a]
                                .reshape(512, CF).astype(nbf))

    pos = np.asarray(inputs["ypos"] if axis == 0 else inputs["xpos"], f32n)[b]

    def tables(rope, scal, h_):
        freq = (np.asarray(rope, f32n) * scal).astype(f32n)
        phi = np.einsum("lp,mghp->lmgh", pos, freq)
        return np.cos(phi), np.sin(phi) * sgn

    ci64, si64 = tables(inputs["rope_inv"], _rope_scaling(HI), HI)
    cf64, sf64 = tables(inputs["rope_fl"], _rope_scaling(HF), HF)

    def rep_i(t64):
        t = t64.reshape(64, M * G * HI)
        t = np.concatenate([t, t], axis=1)
        return np.ascontiguousarray(np.concatenate([t, t], axis=0).astype(nbf))

    def rep_f(t64):
        t = t64.reshape(64, M, G, 1, HF)
        t = np.broadcast_to(t, (64, M, G, F, HF)).reshape(64, -1)
        t = np.concatenate([t, t], axis=1)
        return np.ascontiguousarray(np.concatenate([t, t], axis=0).astype(nbf))

    cos_i, sin_i = rep_i(ci64), rep_i(si64)
    cos_f, sin_f = rep_f(cf64), rep_f(sf64)

    mask = np.asarray(inputs["mask"])[b]
    m64 = mask.astype(f32n) if axis == 0 else mask.T.astype(f32n)
    maskrep = np.ascontiguousarray(
        np.concatenate([m64, m64], axis=0).astype(nbf))

    bvrep = np.zeros((768,), f32n)
    bv_inv = np.asarray(inputs["bv_inv"], f32n)[a]
    bv_fl = np.asarray(inputs["bv_fl"], f32n)[a]
    for c in range(2):
        for g in range(G):
            base = c * 384 + g * 192
            bvrep[base:base + 64] = bv_inv[c, g * 64:(g + 1) * 64]
            for f in range(F):
                bvrep[base + 64 + f * 32:base + 64 + (f + 1) * 32] = \
                    bv_fl[c, g * 32:(g + 1) * 32]
    bvrep = np.ascontiguousarray(
        np.broadcast_to(bvrep, (128, 768)).astype(nbf))

    tloc = np.arange(T)
    line, s = tloc // 64, tloc % 64
    pidx = (s * 64 + line if axis == 0 else tloc).astype(np.int32)
    pidx_sb = np.ascontiguousarray(pidx.reshape(32, 128).T)

    return dict(xi_t=xi_t, xf_t=xf_t, wqi=wqi, wqf=wqf, wki=wki, wkf=wkf,
                wvi=wvi, wvf=wvf, wo_i=wo_i, wo_f=wo_f,
                cos_i=cos_i, sin_i=sin_i, cos_f=cos_f, sin_f=sin_f,
                maskrep=maskrep, bvrep=bvrep, pidx=pidx_sb)


# =====================================================================
# Cached executable (axon PJRT path, mirrors bass2jax.run_bass_via_pjrt)
# =====================================================================

_CACHE = {}


def _get_exec():
    if "exec" in _CACHE:
        return _CACHE["exec"]
    nc = build_nc()
    bass2jax.install_neuronx_cc_hook()
    partition_name = nc.partition_id_tensor.name if nc.partition_id_tensor else None
    in_names, out_names, out_avals = [], [], []
    for alloc in nc.m.functions[0].allocations:
        if not isinstance(alloc, mybir.MemoryLocationSet):
            continue
        name = alloc.memorylocations[0].name
        if alloc.kind == "ExternalInput":
            if name != partition_name:
                in_names.append(name)
        elif alloc.kind == "ExternalOutput":
            out_names.append(name)
            out_avals.append(jax.core.ShapedArray(
                tuple(alloc.tensor_shape), mybir.dt.np(alloc.dtype)))
    n_params = len(in_names)
    n_outs = len(out_avals)
    all_in = in_names + out_names
    if partition_name is not None:
        all_in = all_in + [partition_name]

    def _body(*args):
        operands = list(args)
        if partition_name is not None:
            operands.append(bass2jax.partition_id_tensor())
        outs = bass2jax._bass_exec_p.bind(
            *operands, out_avals=tuple(out_avals), in_names=tuple(all_in),
            out_names=tuple(out_names), lowering_input_output_aliases=(),
            sim_require_finite=False, sim_require_nnan=False, nc=nc)
        return tuple(outs)

    devices = jax.devices()[:N_CORES]
    mesh = Mesh(np.asarray(devices), ("core",))
    in_specs = (PartitionSpec("core"),) * (n_params + n_outs)
    out_specs = (PartitionSpec("core"),) * n_outs
    donate = tuple(range(n_params, n_params + n_outs))
    sharded = jax.jit(
        shard_map(_body, mesh=mesh, in_specs=in_specs, out_specs=out_specs,
                  check_rep=False),
        donate_argnums=donate, keep_unused=True)
    sh = NamedSharding(mesh, PartitionSpec("core"))
    zero_fns = [
        jax.jit(lambda av=av: jax.numpy.zeros((N_CORES * av.shape[0],) + av.shape[1:],
                                              av.dtype), out_shardings=sh)
        for av in out_avals
    ]
    _CACHE["exec"] = (sharded, in_names, out_names, out_avals, mesh, sh, zero_fns)
    return _CACHE["exec"]


def run_cores(per_core_inputs):
    """per_core_inputs: list of 8 dicts name->np array. Returns list of 8
    dicts name->np array."""
    sharded, in_names, out_names, out_avals, mesh, sh, zero_fns = _get_exec()
    concat_in = [
        np.concatenate([np.asarray(per_core_inputs[c][n]) for c in range(N_CORES)],
                       axis=0)
        for n in in_names
    ]
    zeros = [zf() for zf in zero_fns]
    out_arrs = sharded(*concat_in, *zeros)
    res = []
    for c in range(N_CORES):
        res.append({
            name: np.asarray(out_arrs[i]).reshape(N_CORES, *out_avals[i].shape)[c]
            for i, name in enumerate(out_names)})
    return res


# =====================================================================
# kernel() entry point
# =====================================================================

def kernel(x_inv, x_fl, ypos, xpos, mask, Wq_inv, Wq_fl, Wk_inv, Wk_fl,
           Wv_inv, Wv_fl, bv_inv, bv_fl, Wo_inv, Wo_fl, rope_inv, rope_fl):
    inputs = dict(x_inv=x_inv, x_fl=x_fl, ypos=ypos, xpos=xpos, mask=mask,
                  Wq_inv=Wq_inv, Wq_fl=Wq_fl, Wk_inv=Wk_inv, Wk_fl=Wk_fl,
                  Wv_inv=Wv_inv, Wv_fl=Wv_fl, bv_inv=bv_inv, bv_fl=bv_fl,
                  Wo_inv=Wo_inv, Wo_fl=Wo_fl, rope_inv=rope_inv,
                  rope_fl=rope_fl)
    per_core = [host_prep(inputs, c) for c in range(N_CORES)]
    res = run_cores(per_core)
    outs = []
    for b in range(B):
        quarters = [res[4 * b + q]["out"].astype(np.float32) for q in range(4)]
        outs.append(np.concatenate(quarters, axis=0).reshape(Y, X, 1536))
    return np.stack(outs).astype(np.float32)
